# revision 30
# baseline (speedup 1.0000x reference)
"""Trainium2 Bass kernel for 16-head MultiHeadAttention (B=2, S=2048, D=1024, f32).

Sharding: 8 cores = 2 (batch) x 4 (head groups of 4 heads).
Each core gets col-shards of Wq/Wk/Wv ([1024,256]) + a row-shard of Wo
([256,1024]), computes a full [2048,1024] partial output; the host sums the
8 partials (4 per batch element) into [2,2048,1024].

All device data is bf16 (f32 accumulation in PSUM); the host converts inputs
and upconverts the bf16 partial outputs. rel-err budget 2e-2 >> bf16 noise.

On-device pipeline (per core):
  KT/QT = W^T @ x^T                    -> [128(=2 heads x 64), 2048] bf16
  V     = xv^T-tiles @ Wv directly     -> v_sb[j, jt, head, 65] (65th col = 1
          so the AV matmul emits softmax denominators for free)
  sT    = KT_h^T-slice @ QT_h-slice, two heads row-packed in the PE array
          via tile_position (0,0)/(64,0)          [128 j, 2, 512 q] PSUM f32
  expT  = exp(0.125 * sT) on ACT -> bf16 SBUF (ACT is the pacing engine:
          128 instrs x ~1.15us; everything else hides behind it)
  AV    : transposed orientation: lhsT = expT[j, q-slice], rhs = V_aug[j, 65]
          -> out[q, 65] PSUM, full 128x128 PE occupancy (2x fewer cycles than
          the dk-partition orientation). Column 64 = softmax denominator.
  norm  : per-partition reciprocal + tensor_scalar multiply (DVE)
  outT  : PE-transpose of normalized out back to [dg, q] for the Wo matmul
  out  += outT_p^T @ Wo_p accumulated over BOTH head pairs (K=256) -> one
          [2048, 1024] bf16 partial per core (half the output traffic).
"""

import sys

import numpy as np

if "/opt/trn_rl_repo" not in sys.path:
    sys.path.insert(0, "/opt/trn_rl_repo")

import ml_dtypes

import concourse.bacc as bacc
import concourse.mybir as mybir
import concourse.tile as tile
from concourse.masks import make_identity

F32 = mybir.dt.float32
BF16 = mybir.dt.bfloat16

B, S, D, H = 2, 2048, 1024, 16
DK = D // H          # 64
HL = 4               # heads per core
DG = HL * DK         # 256
SCALE = 0.125        # 1/sqrt(DK)

ET = D // 128        # 8 e-tiles (contraction over D)
JT = S // 128        # 16 j-tiles (keys)
QC = S // 512        # 4 q-chunks


def _build_nc():
    nc = bacc.Bacc("TRN2", target_bir_lowering=False, debug=False)

    xq = nc.dram_tensor("xq", [D, S], BF16, kind="ExternalInput").ap()
    xk = nc.dram_tensor("xk", [D, S], BF16, kind="ExternalInput").ap()
    xv = nc.dram_tensor("xv", [D, S], BF16, kind="ExternalInput").ap()
    wq = nc.dram_tensor("wq", [D, DG], BF16, kind="ExternalInput").ap()
    wk = nc.dram_tensor("wk", [D, DG], BF16, kind="ExternalInput").ap()
    wv = nc.dram_tensor("wv", [D, DG], BF16, kind="ExternalInput").ap()
    wo = nc.dram_tensor("wo", [DG, D], BF16, kind="ExternalInput").ap()
    out = nc.dram_tensor("out", [S, D], BF16, kind="ExternalOutput").ap()

    with tile.TileContext(nc) as tc:
        with (
            tc.tile_pool(name="wpool", bufs=1) as wpool,
            tc.tile_pool(name="xin", bufs=1) as xin,
            tc.tile_pool(name="proj", bufs=1) as proj,
            tc.tile_pool(name="expp", bufs=32) as expp,
            tc.tile_pool(name="nrm", bufs=6) as nrm,
            tc.tile_pool(name="osbp", bufs=2) as osbp,
        ):
            # ---- ACT warmup: force the Exp table load at t=0 --------------
            wu_in = wpool.tile([128, 16], F32, tag="wu", name="wu_in")
            nc.vector.memset(wu_in, 0.0)
            wu_out = wpool.tile([128, 16], BF16, tag="wuo", name="wu_out")
            nc.scalar.activation(
                out=wu_out, in_=wu_in,
                func=mybir.ActivationFunctionType.Exp, scale=1.0,
            )

            # ---- constants + weight tiles ---------------------------------
            wk_sb = [wpool.tile([128, DG], BF16, tag=f"wk{e}", name=f"wk{e}")
                     for e in range(ET)]
            wq_sb = [wpool.tile([128, DG], BF16, tag=f"wq{e}", name=f"wq{e}")
                     for e in range(ET)]
            wv_sb = [wpool.tile([128, DG], BF16, tag=f"wv{e}", name=f"wv{e}")
                     for e in range(ET)]
            wo_sb = [wpool.tile([128, D], BF16, tag=f"wo{p}", name=f"wo{p}")
                     for p in range(2)]

            ident_f = wpool.tile([128, 128], F32, tag="ident_f", name="ident_f")
            make_identity(nc, ident_f)
            ident = wpool.tile([128, 128], BF16, tag="ident", name="ident")
            nc.vector.tensor_copy(ident, ident_f)

            # ---- persistent activation tiles ------------------------------
            kt_sb = [proj.tile([128, S], BF16, tag=f"kt{p}", name=f"kt{p}")
                     for p in range(2)]
            qt_sb = [proj.tile([128, S], BF16, tag=f"qt{p}", name=f"qt{p}")
                     for p in range(2)]
            v_sb = proj.tile([128, JT, HL, DK + 1], BF16, tag="v", name="v_sb")
            nc.vector.memset(v_sb[:, :, :, DK:DK + 1], 1.0)
            outt_sb = [proj.tile([128, S], BF16, tag=f"ot{p}", name=f"outt{p}")
                       for p in range(2)]

            # ---- DMA emission (3 queues round-robin) ----------------------
            # Order tuned so exp(c0) can start ~9.5us in: wk, xk[c0], wq,
            # xq[c0], xk[c1..c3] (keeps exp c0 fed), xv[c0], wv, then the
            # rest column-interleaved, wo last.
            queues = [nc.sync, nc.gpsimd]
            rr = [0]

            def dq(dst, src):
                queues[rr[0] % 2].dma_start(dst, src)
                rr[0] += 1

            xk_t = [xin.tile([128, S], BF16, tag=f"xk{e}", name=f"xk{e}")
                    for e in range(ET)]
            xq_t = [xin.tile([128, S], BF16, tag=f"xq{e}", name=f"xq{e}")
                    for e in range(ET)]
            # xv reuses xk's SBUF slots: xk's last readers (K-p1) run before
            # xv's transfers reach the queue head, so this costs no time.
            xv_t = [xin.tile([128, S], BF16, tag=f"xk{e}", name=f"xv{e}")
                    for e in range(ET)]

            def dx(ts, dram, c):
                sl = slice(c * 512, (c + 1) * 512)
                for e in range(ET):
                    dq(ts[e][:, sl], dram[e * 128:(e + 1) * 128, sl])

            for e in range(ET):
                dq(wk_sb[e], wk[e * 128:(e + 1) * 128, :])
            dx(xk_t, xk, 0)
            dx(xk_t, xk, 1)
            for e in range(ET):
                dq(wq_sb[e], wq[e * 128:(e + 1) * 128, :])
            dx(xq_t, xq, 0)
            dx(xk_t, xk, 2)
            dx(xk_t, xk, 3)
            for e in range(ET):
                dq(wv_sb[e], wv[e * 128:(e + 1) * 128, :])
            dx(xv_t, xv, 0)
            dx(xv_t, xv, 1)
            dx(xq_t, xq, 1)
            dx(xv_t, xv, 2)
            dx(xv_t, xv, 3)
            dx(xq_t, xq, 2)
            dx(xq_t, xq, 3)
            for p in range(2):
                dq(wo_sb[p], wo[p * 128:(p + 1) * 128, :])

            # ---- phase A1: just enough for exp(c0) to start ---------------
            # The pool boundary below is a full barrier for successor pools,
            # so ps_a1 holds ONLY K-p0 (all S) + Q-p0-c0: it drains as soon
            # as xk + xq[c0] land (~22us) instead of after all of phase A.
            with tc.tile_pool(name="ps_a1", bufs=1, space="PSUM") as ps_a1:
                for nm, c in (("k", 0), ("q", 0)):
                    w_sb, x_t, dst = {
                        "k": (wk_sb, xk_t, kt_sb), "q": (wq_sb, xq_t, qt_sb),
                    }[nm]
                    csl = slice(c * 512, (c + 1) * 512)
                    acc = ps_a1.tile([128, 512], F32, tag="paq", bufs=2,
                                     name=f"a{nm}0{c}")
                    for e in range(ET):
                        nc.tensor.matmul(
                            acc, w_sb[e][:, 0:128], x_t[e][:, csl],
                            start=(e == 0), stop=(e == ET - 1),
                        )
                    nc.vector.tensor_copy(dst[0][:, csl], acc)

            # ---- phase B: attention + output projection -------------------
            psacc = {}

            def tail(p, c, ex_tiles):
                """AV waves + normalize + transpose (+ Wo when p==1)."""
                ps_acc = psacc["p"]
                hA, hB = 2 * p, 2 * p + 1
                for k in range(4):          # one 128-query tile per wave
                    qsl = slice(k * 128, (k + 1) * 128)
                    accA = ps_acc.tile([128, DK + 1], F32, tag="acc",
                                       name=f"avA{p}{c}{k}")
                    accB = ps_acc.tile([128, DK + 1], F32, tag="acc",
                                       name=f"avB{p}{c}{k}")
                    for jt in range(JT):
                        nc.tensor.matmul(
                            accA, ex_tiles[jt][:, 0, qsl], v_sb[:, jt, hA, :],
                            start=(jt == 0), stop=(jt == JT - 1),
                        )
                        nc.tensor.matmul(
                            accB, ex_tiles[jt][:, 1, qsl], v_sb[:, jt, hB, :],
                            start=(jt == 0), stop=(jt == JT - 1),
                        )
                    recA = nrm.tile([128, 1], F32, tag="rec", name=f"rA{p}{c}{k}")
                    recB = nrm.tile([128, 1], F32, tag="rec", name=f"rB{p}{c}{k}")
                    nc.vector.reciprocal(recA, accA[:, DK:DK + 1])
                    nc.vector.reciprocal(recB, accB[:, DK:DK + 1])
                    nt = nrm.tile([128, 2, DK], BF16, tag="nt", name=f"nt{p}{c}{k}")
                    nc.vector.tensor_scalar(
                        nt[:, 0, :], accA[:, 0:DK], recA, None,
                        mybir.AluOpType.mult)
                    nc.vector.tensor_scalar(
                        nt[:, 1, :], accB[:, 0:DK], recB, None,
                        mybir.AluOpType.mult)
                    pt = ps_acc.tile([128, 128], BF16, tag="acc",
                                     name=f"pt{p}{c}{k}")
                    nc.tensor.transpose(pt, nt.rearrange("q h d -> q (h d)"),
                                        ident)
                    qg = c * 4 + k
                    nc.vector.tensor_copy(
                        outt_sb[p][:, qg * 128:(qg + 1) * 128], pt)
                if p == 1:
                    for k in range(4):
                        qg = c * 4 + k
                        osb = osbp.tile([128, D], BF16, tag="osb",
                                        name=f"osb{c}{k}")
                        for ch in range(2):
                            chsl = slice(ch * 512, (ch + 1) * 512)
                            acc = ps_acc.tile([128, 512], F32, tag="acc",
                                              name=f"po{c}{k}{ch}")
                            nc.tensor.matmul(
                                acc, outt_sb[0][:, qg * 128:(qg + 1) * 128],
                                wo_sb[0][:, chsl], start=True, stop=False)
                            nc.tensor.matmul(
                                acc, outt_sb[1][:, qg * 128:(qg + 1) * 128],
                                wo_sb[1][:, chsl], start=False, stop=True)
                            nc.vector.tensor_copy(osb[:, chsl], acc)
                        nc.sync.dma_start(
                            out[qg * 128:(qg + 1) * 128, :], osb)

            # ps_sc (banks 0-3, after ps_a1's barrier) runs the score ring
            # while ps_a2 (banks 4-7) finishes the REST of phase A under the
            # first two chunks' exp windows; ps_a2 then closes and ps_acc
            # (AV/transpose/Wo ring) takes over its banks.
            sc_cm = tc.tile_pool(name="ps_sc", bufs=2, space="PSUM")
            a2_cm = tc.tile_pool(name="ps_a2", bufs=4, space="PSUM")
            ps_sc = sc_cm.__enter__()
            ps_a2 = a2_cm.__enter__()

            def a2_kq(nm, p, cs):
                w_sb, x_t, dst = {
                    "k": (wk_sb, xk_t, kt_sb), "q": (wq_sb, xq_t, qt_sb),
                }[nm]
                for c in cs:
                    csl = slice(c * 512, (c + 1) * 512)
                    acc = ps_a2.tile([128, 512], F32, tag="pal",
                                     name=f"a{nm}{p}{c}")
                    for e in range(ET):
                        nc.tensor.matmul(
                            acc, w_sb[e][:, p * 128:(p + 1) * 128],
                            x_t[e][:, csl],
                            start=(e == 0), stop=(e == ET - 1))
                    nc.vector.tensor_copy(dst[p][:, csl], acc)

            def a2_v(jts):
                for jt in jts:
                    jsl = slice(jt * 128, (jt + 1) * 128)
                    accv = ps_a2.tile([128, DG], F32, tag="pal",
                                      name=f"av_{jt}")
                    for e in range(ET):
                        nc.tensor.matmul(
                            accv, xv_t[e][:, jsl], wv_sb[e],
                            start=(e == 0), stop=(e == ET - 1))
                    nc.vector.tensor_copy(
                        v_sb[:, jt, :, 0:DK],
                        accv.rearrange("j (h d) -> j h d", h=HL))

            def b_kq(nm, p, cs):
                """Late Q projections on the phase-B acc ring."""
                w_sb, x_t, dst = {
                    "k": (wk_sb, xk_t, kt_sb), "q": (wq_sb, xq_t, qt_sb),
                }[nm]
                for c in cs:
                    csl = slice(c * 512, (c + 1) * 512)
                    acc = psacc["p"].tile([128, 512], F32, tag="acc",
                                          name=f"b{nm}{p}{c}")
                    for e in range(ET):
                        nc.tensor.matmul(
                            acc, w_sb[e][:, p * 128:(p + 1) * 128],
                            x_t[e][:, csl],
                            start=(e == 0), stop=(e == ET - 1))
                    nc.vector.tensor_copy(dst[p][:, csl], acc)

            pend = []
            for i in range(8):
                p, c = i // 4, i % 4
                csl = slice(c * 512, (c + 1) * 512)
                ex_tiles = []
                for jt in range(JT):
                    # K-p0 c1..c3 land just before the scores that read them
                    # (kt columns jt*128 onward), off the ps_a1 barrier path.
                    if i == 0 and jt in (4, 8, 12):
                        a2_kq("k", 0, [jt // 4])
                    jsl = slice(jt * 128, (jt + 1) * 128)
                    sc = ps_sc.tile([128, 2, 512], F32, tag="sc",
                                    name=f"sc{p}{c}{jt}")
                    nc.tensor.matmul(
                        sc[:, 0, :], kt_sb[p][0:64, jsl],
                        qt_sb[p][0:64, csl],
                        start=True, stop=True, tile_position=(0, 0))
                    nc.tensor.matmul(
                        sc[:, 1, :], kt_sb[p][64:128, jsl],
                        qt_sb[p][64:128, csl],
                        start=True, stop=True, tile_position=(64, 0))
                    ex = expp.tile([128, 2, 512], BF16, tag="ex",
                                   name=f"ex{p}{c}{jt}")
                    nc.scalar.activation(
                        out=ex, in_=sc,
                        func=mybir.ActivationFunctionType.Exp,
                        scale=SCALE)
                    ex_tiles.append(ex)
                pend.append((p, c, ex_tiles))
                if i == 0:
                    a2_kq("q", 0, [1])
                    a2_kq("k", 1, range(QC))
                    a2_v(range(0, 8))
                elif i == 1:
                    a2_v(range(8, 16))
                    a2_cm.__exit__(None, None, None)
                    acc_cm = tc.tile_pool(name="ps_acc", bufs=4,
                                          space="PSUM")
                    psacc["p"] = acc_cm.__enter__()
                    b_kq("q", 0, [2])
                elif i == 2:
                    b_kq("q", 0, [3])
                    b_kq("q", 1, [0])
                elif i == 3:
                    b_kq("q", 1, [1, 2])
                elif i == 4:
                    b_kq("q", 1, [3])
                if "p" in psacc:
                    while len(pend) > 1:
                        tail(*pend.pop(0))
            while pend:
                tail(*pend.pop(0))
            acc_cm.__exit__(None, None, None)
            sc_cm.__exit__(None, None, None)

    nc.compile()
    return nc


_NC = None


def _get_nc():
    global _NC
    if _NC is None:
        _NC = _build_nc()
    return _NC


def make_in_maps(query, key, value, Wq, Wk, Wv, Wo):
    bf = ml_dtypes.bfloat16
    xqT = [np.ascontiguousarray(np.asarray(query[b], dtype=np.float32).T.astype(bf))
           for b in range(B)]
    xkT = [np.ascontiguousarray(np.asarray(key[b], dtype=np.float32).T.astype(bf))
           for b in range(B)]
    xvT = [np.ascontiguousarray(np.asarray(value[b], dtype=np.float32).T.astype(bf))
           for b in range(B)]
    Wq = np.asarray(Wq, dtype=np.float32)
    Wk = np.asarray(Wk, dtype=np.float32)
    Wv = np.asarray(Wv, dtype=np.float32)
    Wo = np.asarray(Wo, dtype=np.float32)

    in_maps = []
    for core in range(8):
        b, g = divmod(core, 4)
        sl = slice(g * DG, (g + 1) * DG)
        in_maps.append({
            "xq": xqT[b],
            "xk": xkT[b],
            "xv": xvT[b],
            "wq": np.ascontiguousarray(Wq[:, sl].astype(bf)),
            "wk": np.ascontiguousarray(Wk[:, sl].astype(bf)),
            "wv": np.ascontiguousarray(Wv[:, sl].astype(bf)),
            "wo": np.ascontiguousarray(Wo[sl, :].astype(bf)),
        })
    return in_maps


def combine_results(results):
    out = np.zeros((B, S, D), dtype=np.float32)
    for core in range(8):
        out[core // 4] += results[core]["out"].astype(np.float32)
    return out


def kernel(query, key, value, Wq, Wk, Wv, Wo, _trace=False):
    from concourse import bass_utils

    nc = _get_nc()
    in_maps = make_in_maps(query, key, value, Wq, Wk, Wv, Wo)
    r = bass_utils.run_bass_kernel_spmd(
        nc, in_maps, core_ids=list(range(8)), trace=_trace
    )
    kernel.last_results = r
    return combine_results(r.results)
